# revision 21
# baseline (speedup 1.0000x reference)
"""Trainium2 Bass kernel for nn_ClassConditionalAffinity.

Problem (hardcoded shapes): B=4, D=256, H=W=64, grid=16 -> HW=4096.
Valid pairs are the 4-neighbors of the 16x16 grid of pixels (0,4,...,60)^2
(manhattan distance 4 <= 5), giving 960 directed pairs per batch. The
output A is (B, 4096, 4096): identity everywhere except the 256 grid rows,
which carry up to 4 sigmoid(MLP) affinities at columns row+-4 / row+-256,
then every row is normalized by its sum.

Sharding: 8 cores = 4 batches x 2 row-halves (2048 rows each). Every core
runs the SAME program; per-core differences are carried by the data:
  - features/embeddings are passed as a 10-grid-row halo window (8 own grid
    rows + north/south halo rows, zero-padded at the outer boundary),
  - boundary masks zero out the nonexistent north/south neighbor vals,
  - the upper-half cores write columns rotated by -2048 (mod 4096); the
    host un-rotates with np.roll. This makes every DMA offset a compile
    time constant shared by all 8 cores.

Perf structure (the kernel is a pure HBM-write problem: 33.55 MB/core at
~420 GB/s/core effective across the 16 SDMA engines; the NEFF wrapper adds
a fixed ~7 us preamble + ~8 us semaphore-reset epilogue around the body):
  - ALL device inputs ride in ONE packed [128, 2020] f32 DRAM tensor (w1 |
    w2 | feat | 0.5*emb^T | b1 | b2 | w3) + a [1, 257] mask/b3 row: two
    scalar-queue HWDGE DMAs total, so the output write stream on the sync
    queue never stalls on the 8-entry DMAHW semaphore rotation (the old
    10-load version wedged the sync engine for ~3.5 us at the head). The
    embedding transpose and 0.5 scale are folded on the host, which also
    removes the PE-transpose + PSUM round trip.
  - A single [128, 4096] SBUF tile iz = [identity | zeros] sources every
    zero/identity region; each 128-row block needs at most 2 DMAs. The
    memsets are split gpsimd/vector, identity first, ordered so each
    region is ready just before the stream ordering below consumes it;
    the first output DMA issues ~1 us after the framework preamble ends.
  - Stream order interleaves narrow leads (earliest-ready sources) with
    ~1-2 MB ident|zeros bodies so the SDMA in-flight window (8 rotating
    completion sems) always holds several MB of queued work.
  - Everything that depends on the MLP result (the 8 even-block 640-wide
    patch windows, 2.6 MB total) is emitted last, so the list scheduler
    puts it at the tail of the sync queue: the write stream never stalls
    waiting for compute. The MLP finishes ~25 us into an ~83 us stream.

Device program per core:
  1. DMA the masks row then the packed input block (scalar HWDGE queue).
  2. Assemble xT (640 x 512) for the 496 local pairs (4 neighbor classes)
     with strided DVE copies, then run the 3-layer MLP on PE with
     transposed activations (no inter-layer transposes needed), biases and
     relu/sigmoid fused on the scalar engine.
  3. Row sums + reciprocal on DVE; scatter the scaled values into a tiny
     (128 x 8 x 5) per-partition table V via 5 small SBUF->SBUF DMAs.
  4. Build the 8 patch tiles (5 shifted diagonals scaled by V) and write
     them into the deferred 640-wide windows.
"""

import os
import numpy as np

import concourse.bass as bass
import concourse.mybir as mybir
import concourse.tile as tile
from concourse import bacc
from concourse.bass_utils import run_bass_kernel_spmd
from concourse.masks import make_identity

F32 = mybir.dt.float32
BF16 = mybir.dt.bfloat16
AF = mybir.ActivationFunctionType

B, D, H, W = 4, 256, 64, 64
HW = H * W                      # 4096
G = 16                          # grid points per axis
TG = 8                          # own grid rows (gi) per core
ROWS = 2048                     # rows per core shard
NB = 16                         # 128-row blocks per shard
NPAIR = 496                     # E/W: 8*15 each, N/S: 8*16 each
MPAD = 512
MLP_IN, H1, H2 = 640, 256, 128

# packed bf16 input column layout: w1 | w2 | feat | emb^T/2 | w3
# (the MLP runs with bf16 operands + fp32 PSUM accumulation: rel err vs
# the fp32 reference is ~3e-4, far under the 2e-2 gate, and the input
# HBM read drops from 1.03 MB to 0.52 MB)
C_W1, C_W2, C_FT, C_EM = 0, 1280, 1536, 1856
C_W3, NCOL = 2016, 2017

LAST_RESULTS = None             # test.py reads exec_time_ns from here


def _build_nc():
    nc = bacc.Bacc("TRN2", target_bir_lowering=False)

    inpd = nc.dram_tensor("inp", [128, NCOL], BF16, kind="ExternalInput")
    b12d = nc.dram_tensor("b12", [128, 3], F32, kind="ExternalInput")
    mskd = nc.dram_tensor("msk", [1, 257], F32, kind="ExternalInput")
    a = nc.dram_tensor("a", [ROWS, HW], F32, kind="ExternalOutput")

    from contextlib import ExitStack

    with tile.TileContext(nc) as tc, ExitStack() as ctx:
        consts = ctx.enter_context(tc.tile_pool(name="consts", bufs=1))
        psum = ctx.enter_context(tc.tile_pool(name="psum", bufs=1, space="PSUM"))

        # ---- the two input loads (scalar HWDGE queue). Masks first: its
        # completion sem recycles almost immediately, so when the DMAHW
        # rotation wraps around to it the write stream never waits. The
        # input load drains inside the ~5 us post-launch DMA ramp window
        # (the write stream only reaches full rate at ~13.5 us regardless
        # of configuration, so the input reads ride along for free). ----
        inp = consts.tile([128, NCOL], BF16)
        nc.scalar.dma_start(out=inp, in_=inpd[:])
        b12 = consts.tile([128, 3], F32)
        nc.scalar.dma_start(out=b12, in_=b12d[:])
        mskt = consts.tile([1, 257], F32)
        nc.scalar.dma_start(out=mskt, in_=mskd[:])

        # ---- stream constants: iz = [identity(128) | zeros(3968)] ----
        # identity first (every body DMA needs it), then zeros split
        # gpsimd/vector in the order the stream below consumes them.
        iz = consts.tile([128, HW], F32)
        make_identity(nc, iz[:, 0:128])
        nc.gpsimd.memset(iz[:, 128:256], 0.0)
        nc.gpsimd.memset(iz[:, 256:640], 0.0)
        nc.vector.memset(iz[:, 640:1792], 0.0)
        nc.gpsimd.memset(iz[:, 1792:2432], 0.0)
        nc.vector.memset(iz[:, 2432:3456], 0.0)
        nc.gpsimd.memset(iz[:, 3456:4096], 0.0)
        ident = iz[:, 0:128]

        # ---- the v-independent output stream (sync queue) ----
        # odd block lb: lead zeros [0:c0) + body [c0:4096) = ident|zeros
        # even block lb: lead zeros [0:w0) + trail zeros [w0+640:4096),
        #   the 640-wide patch window [w0, w0+640) is written later.
        def rows_of(lb):
            return a[128 * lb : 128 * (lb + 1), :]

        def lead(lb, w):
            nc.sync.dma_start(out=rows_of(lb)[:, 0:w], in_=iz[:, 128 : 128 + w])

        def body(lb):
            c0 = 128 * lb
            nc.sync.dma_start(out=rows_of(lb)[:, c0:HW], in_=iz[:, 0 : HW - c0])

        def trail(lb):
            w0 = 128 * lb - 256
            wz = HW - w0 - 640
            nc.sync.dma_start(
                out=rows_of(lb)[:, w0 + 640 : HW], in_=iz[:, 128 : 128 + wz]
            )

        # Order: narrow leads first (their zero sources are memset
        # earliest) interleaved with the big ident|zeros bodies (odd
        # blocks only) so the hardware's in-flight DMA window always
        # holds several MB of queued work. Wide trails/mid go once the
        # full iz is ready.
        lead(1, 128)
        lead(4, 256)
        body(15)
        lead(3, 384)
        body(13)
        lead(6, 512)
        body(11)
        lead(5, 640)
        body(9)
        lead(8, 768)
        body(7)
        lead(7, 896)
        trail(14)
        lead(10, 1024)
        trail(12)
        lead(9, 1152)
        trail(10)
        lead(12, 1280)
        trail(8)
        lead(11, 1408)
        trail(6)
        lead(14, 1536)
        trail(4)
        lead(13, 1664)
        trail(2)  # w0=0: only the trailing zeros [640:4096)
        lead(15, 1920)
        # lb=0: patch window wraps: [3840:4096) + [0:384), zeros between
        nc.sync.dma_start(out=rows_of(0)[:, 384:3840], in_=iz[:, 128 : 128 + 3456])
        body(5)

        # ---- views into the packed input block ----
        w1v = inp[:, C_W1:C_W2].rearrange("p (k n) -> p k n", n=H1)
        w2v = inp[:, C_W2:C_FT].rearrange("p (k n) -> p k n", n=H2)
        g0 = inp[:, C_FT : C_FT + 160].rearrange("p (t g) -> p t g", g=G)
        g1 = inp[:, C_FT + 160 : C_EM].rearrange("p (t g) -> p t g", g=G)
        embt = inp[:, C_EM:C_W3].rearrange("p (t g) -> p t g", g=G)
        w3sb = inp[:, C_W3:NCOL]
        b1sb = b12[:, 0:2]
        b2sb = b12[:, 2:3]
        mn = mskt[0:1, 0:128]
        ms = mskt[0:1, 128:256]
        b3sb = mskt[0:1, 256:257]

        # ---- assemble xT (640 x 512), pair order: E | W | N | S ----
        xt = [consts.tile([128, MPAD], BF16, name=f"xt{k}") for k in range(5)]
        for k in range(5):
            nc.vector.memset(xt[k][:, NPAIR:MPAD], 0.0)

        # pair storage is (g, t)-major: idx = g*8 + t (t contiguous), so the
        # later per-partition scatter DMAs have a stride-1 inner dim
        def cview(apx, lo, n, gwidth):
            return apx[:, lo : lo + n].rearrange("p (g t) -> p g t", t=TG)

        def gswap(apx):
            return apx.rearrange("p t g -> p g t")

        for ki, gt in ((0, g0), (1, g1)):
            f1a, f2a = xt[ki], xt[ki + 2]
            # E: f1=(t,0:15) f2=(t,1:16)
            nc.vector.tensor_copy(cview(f1a, 0, 120, 15), gswap(gt[:, 1:9, 0:15]))
            nc.vector.tensor_copy(cview(f2a, 0, 120, 15), gswap(gt[:, 1:9, 1:16]))
            # W: f1=(t,1:16) f2=(t,0:15)
            nc.vector.tensor_copy(cview(f1a, 120, 120, 15), gswap(gt[:, 1:9, 1:16]))
            nc.vector.tensor_copy(cview(f2a, 120, 120, 15), gswap(gt[:, 1:9, 0:15]))
            # N: f1=own rows, f2=rows above (halo index t)
            nc.vector.tensor_copy(cview(f1a, 240, 128, 16), gswap(gt[:, 1:9, :]))
            nc.vector.tensor_copy(cview(f2a, 240, 128, 16), gswap(gt[:, 0:8, :]))
            # S: f2=rows below (halo index t+2)
            nc.vector.tensor_copy(cview(f1a, 368, 128, 16), gswap(gt[:, 1:9, :]))
            nc.vector.tensor_copy(cview(f2a, 368, 128, 16), gswap(gt[:, 2:10, :]))
        # coord rows: 0.5*(emb[p1]+emb[p2]) with the 0.5 folded on host
        ct = xt[4]
        nc.vector.tensor_add(cview(ct, 0, 120, 15), gswap(embt[:, 1:9, 0:15]), gswap(embt[:, 1:9, 1:16]))
        nc.vector.tensor_add(cview(ct, 120, 120, 15), gswap(embt[:, 1:9, 1:16]), gswap(embt[:, 1:9, 0:15]))
        nc.vector.tensor_add(cview(ct, 240, 128, 16), gswap(embt[:, 1:9, :]), gswap(embt[:, 0:8, :]))
        nc.vector.tensor_add(cview(ct, 368, 128, 16), gswap(embt[:, 1:9, :]), gswap(embt[:, 2:10, :]))

        # ---- MLP (transposed activations) ----
        h1sb = consts.tile([128, 2, MPAD], BF16)
        for n in range(2):
            ps1 = psum.tile([128, MPAD], F32)
            for k in range(5):
                nc.tensor.matmul(
                    ps1,
                    w1v[:, k, 128 * n : 128 * (n + 1)],
                    xt[k][:],
                    start=(k == 0),
                    stop=(k == 4),
                )
            nc.scalar.activation(h1sb[:, n, :], ps1, AF.Relu, bias=b1sb[:, n : n + 1])
        ps2 = psum.tile([128, MPAD], F32)
        for k in range(2):
            nc.tensor.matmul(ps2, w2v[:, k, :], h1sb[:, k, :], start=(k == 0), stop=(k == 1))
        h2sb = consts.tile([128, MPAD], BF16)
        nc.scalar.activation(h2sb, ps2, AF.Relu, bias=b2sb[:, 0:1])
        ps3 = psum.tile([1, MPAD], F32)
        nc.tensor.matmul(ps3, w3sb[:], h2sb[:], start=True, stop=True)
        vals = consts.tile([1, MPAD], F32)
        nc.scalar.activation(vals, ps3, AF.Sigmoid, bias=b3sb)

        # ---- row sums, reciprocal, scaled values ----
        vnm = consts.tile([1, 128], F32)
        vsm = consts.tile([1, 128], F32)
        nc.vector.tensor_mul(vnm, vals[:, 240:368], mn)
        nc.vector.tensor_mul(vsm, vals[:, 368:496], ms)

        s = consts.tile([1, 128], F32)
        nc.vector.memset(s, 1.0)
        s3 = s.rearrange("o (g t) -> o g t", t=TG)
        nc.vector.tensor_add(s3[:, 0:15, :], s3[:, 0:15, :], cview(vals, 0, 120, 15))
        nc.vector.tensor_add(s3[:, 1:16, :], s3[:, 1:16, :], cview(vals, 120, 120, 15))
        nc.vector.tensor_add(s, s, vnm[:])
        nc.vector.tensor_add(s, s, vsm[:])
        recip = consts.tile([1, 128], F32)
        nc.vector.reciprocal(recip, s)
        r3 = recip.rearrange("o (g t) -> o g t", t=TG)

        ve = consts.tile([1, 120], F32)
        vw = consts.tile([1, 120], F32)
        vn = consts.tile([1, 128], F32)
        vs = consts.tile([1, 128], F32)
        nc.vector.tensor_mul(cview(ve, 0, 120, 15), cview(vals, 0, 120, 15), r3[:, 0:15, :])
        nc.vector.tensor_mul(cview(vw, 0, 120, 15), cview(vals, 120, 120, 15), r3[:, 1:16, :])
        nc.vector.tensor_mul(vn, vnm[:], recip[:])
        nc.vector.tensor_mul(vs, vsm[:], recip[:])

        # ---- V table: (128 partitions) x (5 offsets) x (8 blocks) ----
        # offsets: 0:-256(N) 1:-4(W) 2:diag 3:+4(E) 4:+256(S)
        v = consts.tile([128, 5, TG], F32)
        nc.vector.memset(v, 0.0)
        nc.vector.memset(v[:, 2, :], 1.0)
        with nc.allow_non_contiguous_dma(reason="tiny per-partition scatter"):
            nc.gpsimd.dma_start(out=v[0:61:4, 2, :], in_=r3[:])
            nc.gpsimd.dma_start(
                out=v[0:61:4, 0, :], in_=vn.rearrange("o (g t) -> o g t", t=TG)
            )
            nc.gpsimd.dma_start(
                out=v[0:61:4, 4, :], in_=vs.rearrange("o (g t) -> o g t", t=TG)
            )
            nc.gpsimd.dma_start(
                out=v[0:57:4, 3, :], in_=ve.rearrange("o (g t) -> o g t", t=TG)
            )
            nc.gpsimd.dma_start(
                out=v[4:61:4, 1, :], in_=vw.rearrange("o (g t) -> o g t", t=TG)
            )

        # ---- build all 8 patch tiles at once (5 shifted diagonals scaled
        # by V) with stride-0 broadcast APs: 7 wide DVE ops (~4us) instead
        # of 64 per-tile ops, so pall is ready right after the MLP ----
        pall = consts.tile([128, TG, 640], F32)
        tmp = consts.tile([128, TG, 128], F32)
        nc.vector.memset(pall[:, :, 128:512], 0.0)
        ib = ident.unsqueeze(1).broadcast_to((128, TG, 128))

        def vb(k):
            return v[:, k, :].unsqueeze(2).broadcast_to((128, TG, 128))

        nc.vector.tensor_mul(pall[:, :, 0:128], ib, vb(0))
        nc.vector.tensor_mul(pall[:, :, 512:640], ib, vb(4))
        nc.vector.tensor_mul(pall[:, :, 252:380], ib, vb(1))
        nc.vector.tensor_mul(tmp, ib, vb(2))
        nc.vector.tensor_add(pall[:, :, 256:384], pall[:, :, 256:384], tmp[:])
        nc.vector.tensor_mul(tmp, ib, vb(3))
        nc.vector.tensor_add(pall[:, :, 260:388], pall[:, :, 260:388], tmp[:])

        # ---- the deferred patch-window writes (MLP done ~25 us into an
        # ~83 us stream), then the two widest bodies close the stream.
        # The patches are completion-rotation gated (~5 us apart at the
        # tail), so the big bodies MUST come after them: ending the FIFO
        # on 9 rotation-gated small DMAs drains the queue dry and
        # serializes them at ~5 us each (+17 us, measured). ----
        nc.sync.dma_start(out=rows_of(0)[:, 3840:HW], in_=pall[:, 0, 0:256])
        nc.sync.dma_start(out=rows_of(0)[:, 0:384], in_=pall[:, 0, 256:640])
        for t in range(1, TG):
            lb = 2 * t
            w0 = 128 * lb - 256
            nc.sync.dma_start(out=rows_of(lb)[:, w0 : w0 + 640], in_=pall[:, t, :])
        body(3)
        body(1)
    nc.compile()  # bacc register allocation — required before NEFF compile
    return nc


_NC_CACHE = None


def _get_nc():
    global _NC_CACHE
    if _NC_CACHE is None:
        _NC_CACHE = _build_nc()
    return _NC_CACHE


def kernel(**inputs) -> np.ndarray:
    global LAST_RESULTS
    features = np.ascontiguousarray(np.asarray(inputs["features"], dtype=np.float32))
    class_idx = int(np.asarray(inputs["class_idx"]))
    Hv = int(np.asarray(inputs["H"]))
    Wv = int(np.asarray(inputs["W"]))
    gs = int(np.asarray(inputs["grid_size"]))
    assert (Hv, Wv, gs) == (H, W, G), (Hv, Wv, gs)
    emb_table = np.asarray(inputs["emb_table"], dtype=np.float32)
    w1 = np.ascontiguousarray(np.asarray(inputs["W1"], np.float32)[class_idx])
    b1 = np.asarray(inputs["b1"], np.float32)[class_idx]
    w2 = np.ascontiguousarray(np.asarray(inputs["W2"], np.float32)[class_idx])
    b2 = np.asarray(inputs["b2"], np.float32)[class_idx]
    w3 = np.ascontiguousarray(np.asarray(inputs["W3"], np.float32)[class_idx])
    b3 = np.asarray(inputs["b3"], np.float32)[class_idx]

    # grid embeddings: rows gi*64+gj for gi,gj in {0,4,...,60}
    emb4 = np.ascontiguousarray(
        emb_table[: HW].reshape(H, W, 128)[::4, ::4]
    )  # (16,16,128)
    featg = features[:, :, ::4, ::4]  # (B, 256, 16, 16) strided view

    # class-independent part of the packed input block (bf16) + fp32 biases
    import ml_dtypes

    base = np.zeros((128, NCOL), ml_dtypes.bfloat16)
    base[:, C_W1:C_W2] = w1.reshape(5, 128, H1).transpose(1, 0, 2).reshape(128, 1280)
    base[:, C_W2:C_FT] = w2.reshape(2, 128, H2).transpose(1, 0, 2).reshape(128, 256)
    base[:, C_W3] = w3[:, 0]
    b12c = np.zeros((128, 3), np.float32)
    b12c[:, 0:2] = b1.reshape(2, 128).T
    b12c[:, 2] = b2

    in_maps = []
    for c in range(8):
        bb, hh = c // 2, c % 2
        # halo rows: local t=0 is north halo, t=1..8 own, t=9 south halo
        gus = [8 * hh - 1] + list(range(8 * hh, 8 * hh + 8)) + [8 * hh + 8]
        feat_core = np.zeros((D, 10, G), np.float32)
        emb_core = np.zeros((10 * G, 128), np.float32)
        for i, gu in enumerate(gus):
            if 0 <= gu < G:
                feat_core[:, i, :] = featg[bb, :, gu, :]
                emb_core[i * G : (i + 1) * G, :] = emb4[gu]
        inp_core = base.copy()
        inp_core[:, C_FT:C_EM] = (
            feat_core.reshape(2, 128, 160).transpose(1, 0, 2).reshape(128, 320)
        )
        inp_core[:, C_EM:C_W3] = 0.5 * emb_core.T
        msk_core = np.ones((1, 257), np.float32)
        # (g,t)-major: t=0 rows sit at indices g*8+0, t=7 at g*8+7
        if hh == 0:
            msk_core[0, 0:128:8] = 0.0  # no north neighbor on grid row 0
        else:
            msk_core[0, 135:256:8] = 0.0  # no south neighbor on grid row 15
        msk_core[0, 256] = b3[0]
        in_maps.append({"inp": inp_core, "b12": b12c, "msk": msk_core})

    nc = _get_nc()
    res = run_bass_kernel_spmd(nc, in_maps, core_ids=list(range(8)))
    LAST_RESULTS = res

    out = np.empty((B, HW, HW), np.float32)
    for c in range(8):
        bb, hh = c // 2, c % 2
        shard = res.results[c]["a"]
        if hh:
            shard = np.roll(shard, 2048, axis=1)
        out[bb, 2048 * hh : 2048 * (hh + 1), :] = shard
    return out


# revision 24
# speedup vs baseline: 1.0187x; 1.0187x over previous
"""Trainium2 Bass kernel for nn_ClassConditionalAffinity.

Problem (hardcoded shapes): B=4, D=256, H=W=64, grid=16 -> HW=4096.
Valid pairs are the 4-neighbors of the 16x16 grid of pixels (0,4,...,60)^2
(manhattan distance 4 <= 5), giving 960 directed pairs per batch. The
output A is (B, 4096, 4096): identity everywhere except the 256 grid rows,
which carry up to 4 sigmoid(MLP) affinities at columns row+-4 / row+-256,
then every row is normalized by its sum.

Sharding: 8 cores = 4 batches x 2 row-halves (2048 rows each). Every core
runs the SAME program; per-core differences are carried by the data:
  - features/embeddings are passed as a 10-grid-row halo window (8 own grid
    rows + north/south halo rows, zero-padded at the outer boundary),
  - boundary masks zero out the nonexistent north/south neighbor vals,
  - the upper-half cores write columns rotated by -2048 (mod 4096); the
    host un-rotates with np.roll. This makes every DMA offset a compile
    time constant shared by all 8 cores.

Perf structure (the kernel is a pure HBM-write problem: 33.55 MB/core at
~425 GB/s/core effective across the 16 SDMA engines; the NEFF wrapper adds
a fixed ~7 us preamble + ~7 us semaphore-reset epilogue around the body):
  - ALL large device inputs ride in ONE packed [128, 2017] BF16 DRAM
    tensor (w1 | w2 | feat | 0.5*emb^T | w3; the MLP runs bf16 operands
    with fp32 PSUM accumulation, rel err ~5e-4 vs the 2e-2 gate) + a
    [128, 3] f32 bias block + a [1, 257] mask/b3 row: three scalar-queue
    HWDGE DMAs total, so the output write stream on the sync queue never
    stalls on the 8-entry DMAHW semaphore rotation (the old 10-load
    version wedged the sync engine for ~3.5 us at the head). The
    embedding transpose and 0.5 scale are folded on the host, which also
    removes the PE-transpose + PSUM round trip. The 0.52 MB input read
    drains inside the ~5 us post-launch DMA ramp window, during which
    HBM-read descriptors would cap the write stream anyway.
  - A single [128, 4096] SBUF tile iz = [identity | zeros] sources every
    zero/identity region; each 128-row block needs at most 2 DMAs. The
    memsets are split gpsimd/vector, identity first, ordered so each
    region is ready just before the stream ordering below consumes it;
    the first output DMA issues ~1 us after the framework preamble ends.
  - Stream order interleaves narrow leads (earliest-ready sources) with
    ~1-2 MB ident|zeros bodies so the SDMA in-flight window (8 rotating
    completion sems) always holds several MB of queued work.
  - Everything that depends on the MLP result (the 8 even-block 640-wide
    patch windows, 2.6 MB total) is emitted last, so the list scheduler
    puts it at the tail of the sync queue: the write stream never stalls
    waiting for compute. The MLP finishes ~25 us into an ~83 us stream.

Device program per core:
  1. DMA the masks row then the packed input block (scalar HWDGE queue).
  2. Assemble xT (640 x 512) for the 496 local pairs (4 neighbor classes)
     with strided DVE copies, then run the 3-layer MLP on PE with
     transposed activations (no inter-layer transposes needed), biases and
     relu/sigmoid fused on the scalar engine.
  3. Row sums + reciprocal on DVE; scatter the scaled values into a tiny
     (128 x 8 x 5) per-partition table V via 5 small SBUF->SBUF DMAs.
  4. Build the 8 patch tiles (5 shifted diagonals scaled by V) and write
     them into the deferred 640-wide windows.
"""

import os
import numpy as np

import concourse.bass as bass
import concourse.mybir as mybir
import concourse.tile as tile
from concourse import bacc
from concourse.bass_utils import run_bass_kernel_spmd
from concourse.masks import make_identity

F32 = mybir.dt.float32
BF16 = mybir.dt.bfloat16
AF = mybir.ActivationFunctionType

B, D, H, W = 4, 256, 64, 64
HW = H * W                      # 4096
G = 16                          # grid points per axis
TG = 8                          # own grid rows (gi) per core
ROWS = 2048                     # rows per core shard
NB = 16                         # 128-row blocks per shard
NPAIR = 496                     # E/W: 8*15 each, N/S: 8*16 each
MPAD = 512
MLP_IN, H1, H2 = 640, 256, 128

# packed bf16 input column layout: w1 | w2 | feat | emb^T/2 | w3
# (the MLP runs with bf16 operands + fp32 PSUM accumulation: rel err vs
# the fp32 reference is ~3e-4, far under the 2e-2 gate, and the input
# HBM read drops from 1.03 MB to 0.52 MB)
C_W1, C_W2, C_FT, C_EM = 0, 1280, 1536, 1856
C_W3, NCOL = 2016, 2017

LAST_RESULTS = None             # test.py reads exec_time_ns from here


def _build_nc():
    nc = bacc.Bacc("TRN2", target_bir_lowering=False)

    inpd = nc.dram_tensor("inp", [128, NCOL], BF16, kind="ExternalInput")
    b12d = nc.dram_tensor("b12", [128, 3], F32, kind="ExternalInput")
    mskd = nc.dram_tensor("msk", [1, 257], F32, kind="ExternalInput")
    a = nc.dram_tensor("a", [ROWS, HW], F32, kind="ExternalOutput")

    from contextlib import ExitStack

    with tile.TileContext(nc) as tc, ExitStack() as ctx:
        consts = ctx.enter_context(tc.tile_pool(name="consts", bufs=1))
        psum = ctx.enter_context(tc.tile_pool(name="psum", bufs=1, space="PSUM"))

        # ---- the two input loads (scalar HWDGE queue). Masks first: its
        # completion sem recycles almost immediately, so when the DMAHW
        # rotation wraps around to it the write stream never waits. The
        # input load drains inside the ~5 us post-launch DMA ramp window
        # (the write stream only reaches full rate at ~13.5 us regardless
        # of configuration, so the input reads ride along for free). ----
        # The packed input is split in two chunks with the tiny loads
        # between them: the gap in read-descriptor arrival gives the SDMA
        # engines a window to serve the write queue mid-ramp (a single
        # 0.52 MB read monopolizes all 16 engines for ~3 us).
        inp = consts.tile([128, NCOL], BF16)
        nc.scalar.dma_start(out=inp[:, 0:1024], in_=inpd[:, 0:1024])
        b12 = consts.tile([128, 3], F32)
        nc.scalar.dma_start(out=b12, in_=b12d[:])
        mskt = consts.tile([1, 257], F32)
        nc.scalar.dma_start(out=mskt, in_=mskd[:])
        nc.scalar.dma_start(out=inp[:, 1024:NCOL], in_=inpd[:, 1024:NCOL])

        # ---- stream constants: iz = [identity(128) | zeros(3968)] ----
        # identity first (every body DMA needs it), then zeros split
        # gpsimd/vector in the order the stream below consumes them.
        iz = consts.tile([128, HW], F32)
        make_identity(nc, iz[:, 0:128])
        nc.gpsimd.memset(iz[:, 128:256], 0.0)
        nc.gpsimd.memset(iz[:, 256:640], 0.0)
        nc.vector.memset(iz[:, 640:1792], 0.0)
        nc.gpsimd.memset(iz[:, 1792:2432], 0.0)
        nc.vector.memset(iz[:, 2432:3456], 0.0)
        nc.gpsimd.memset(iz[:, 3456:4096], 0.0)
        ident = iz[:, 0:128]

        # ---- the v-independent output stream (sync queue) ----
        # odd block lb: lead zeros [0:c0) + body [c0:4096) = ident|zeros
        # even block lb: lead zeros [0:w0) + trail zeros [w0+640:4096),
        #   the 640-wide patch window [w0, w0+640) is written later.
        def rows_of(lb):
            return a[128 * lb : 128 * (lb + 1), :]

        def lead(lb, w):
            nc.sync.dma_start(out=rows_of(lb)[:, 0:w], in_=iz[:, 128 : 128 + w])

        def body(lb):
            c0 = 128 * lb
            nc.sync.dma_start(out=rows_of(lb)[:, c0:HW], in_=iz[:, 0 : HW - c0])

        def trail(lb):
            w0 = 128 * lb - 256
            wz = HW - w0 - 640
            nc.sync.dma_start(
                out=rows_of(lb)[:, w0 + 640 : HW], in_=iz[:, 128 : 128 + wz]
            )

        # Order: narrow leads first (their zero sources are memset
        # earliest) interleaved with the big ident|zeros bodies (odd
        # blocks only) so the hardware's in-flight DMA window always
        # holds several MB of queued work. Wide trails/mid go once the
        # full iz is ready.
        lead(1, 128)
        lead(4, 256)
        body(15)
        lead(3, 384)
        body(13)
        lead(6, 512)
        body(11)
        lead(5, 640)
        body(9)
        lead(8, 768)
        body(7)
        lead(7, 896)
        trail(14)
        lead(10, 1024)
        trail(12)
        lead(9, 1152)
        trail(10)
        lead(12, 1280)
        trail(8)
        lead(11, 1408)
        trail(6)
        lead(14, 1536)
        trail(4)
        lead(13, 1664)
        trail(2)  # w0=0: only the trailing zeros [640:4096)
        lead(15, 1920)
        # lb=0: patch window wraps: [3840:4096) + [0:384), zeros between
        nc.sync.dma_start(out=rows_of(0)[:, 384:3840], in_=iz[:, 128 : 128 + 3456])
        body(5)

        # ---- views into the packed input block ----
        w1v = inp[:, C_W1:C_W2].rearrange("p (k n) -> p k n", n=H1)
        w2v = inp[:, C_W2:C_FT].rearrange("p (k n) -> p k n", n=H2)
        g0 = inp[:, C_FT : C_FT + 160].rearrange("p (t g) -> p t g", g=G)
        g1 = inp[:, C_FT + 160 : C_EM].rearrange("p (t g) -> p t g", g=G)
        embt = inp[:, C_EM:C_W3].rearrange("p (t g) -> p t g", g=G)
        w3sb = inp[:, C_W3:NCOL]
        b1sb = b12[:, 0:2]
        b2sb = b12[:, 2:3]
        mn = mskt[0:1, 0:128]
        ms = mskt[0:1, 128:256]
        b3sb = mskt[0:1, 256:257]

        # ---- assemble xT (640 x 512), pair order: E | W | N | S ----
        xt = [consts.tile([128, MPAD], BF16, name=f"xt{k}") for k in range(5)]
        for k in range(5):
            nc.vector.memset(xt[k][:, NPAIR:MPAD], 0.0)

        # pair storage is (g, t)-major: idx = g*8 + t (t contiguous), so the
        # later per-partition scatter DMAs have a stride-1 inner dim
        def cview(apx, lo, n, gwidth):
            return apx[:, lo : lo + n].rearrange("p (g t) -> p g t", t=TG)

        def gswap(apx):
            return apx.rearrange("p t g -> p g t")

        for ki, gt in ((0, g0), (1, g1)):
            f1a, f2a = xt[ki], xt[ki + 2]
            # E: f1=(t,0:15) f2=(t,1:16)
            nc.vector.tensor_copy(cview(f1a, 0, 120, 15), gswap(gt[:, 1:9, 0:15]))
            nc.vector.tensor_copy(cview(f2a, 0, 120, 15), gswap(gt[:, 1:9, 1:16]))
            # W: f1=(t,1:16) f2=(t,0:15)
            nc.vector.tensor_copy(cview(f1a, 120, 120, 15), gswap(gt[:, 1:9, 1:16]))
            nc.vector.tensor_copy(cview(f2a, 120, 120, 15), gswap(gt[:, 1:9, 0:15]))
            # N: f1=own rows, f2=rows above (halo index t)
            nc.vector.tensor_copy(cview(f1a, 240, 128, 16), gswap(gt[:, 1:9, :]))
            nc.vector.tensor_copy(cview(f2a, 240, 128, 16), gswap(gt[:, 0:8, :]))
            # S: f2=rows below (halo index t+2)
            nc.vector.tensor_copy(cview(f1a, 368, 128, 16), gswap(gt[:, 1:9, :]))
            nc.vector.tensor_copy(cview(f2a, 368, 128, 16), gswap(gt[:, 2:10, :]))
        # coord rows: 0.5*(emb[p1]+emb[p2]) with the 0.5 folded on host
        ct = xt[4]
        nc.vector.tensor_add(cview(ct, 0, 120, 15), gswap(embt[:, 1:9, 0:15]), gswap(embt[:, 1:9, 1:16]))
        nc.vector.tensor_add(cview(ct, 120, 120, 15), gswap(embt[:, 1:9, 1:16]), gswap(embt[:, 1:9, 0:15]))
        nc.vector.tensor_add(cview(ct, 240, 128, 16), gswap(embt[:, 1:9, :]), gswap(embt[:, 0:8, :]))
        nc.vector.tensor_add(cview(ct, 368, 128, 16), gswap(embt[:, 1:9, :]), gswap(embt[:, 2:10, :]))

        # ---- MLP (transposed activations) ----
        h1sb = consts.tile([128, 2, MPAD], BF16)
        for n in range(2):
            ps1 = psum.tile([128, MPAD], F32)
            for k in range(5):
                nc.tensor.matmul(
                    ps1,
                    w1v[:, k, 128 * n : 128 * (n + 1)],
                    xt[k][:],
                    start=(k == 0),
                    stop=(k == 4),
                )
            nc.scalar.activation(h1sb[:, n, :], ps1, AF.Relu, bias=b1sb[:, n : n + 1])
        ps2 = psum.tile([128, MPAD], F32)
        for k in range(2):
            nc.tensor.matmul(ps2, w2v[:, k, :], h1sb[:, k, :], start=(k == 0), stop=(k == 1))
        h2sb = consts.tile([128, MPAD], BF16)
        nc.scalar.activation(h2sb, ps2, AF.Relu, bias=b2sb[:, 0:1])
        ps3 = psum.tile([1, MPAD], F32)
        nc.tensor.matmul(ps3, w3sb[:], h2sb[:], start=True, stop=True)
        vals = consts.tile([1, MPAD], F32)
        nc.scalar.activation(vals, ps3, AF.Sigmoid, bias=b3sb)

        # ---- row sums, reciprocal, scaled values ----
        vnm = consts.tile([1, 128], F32)
        vsm = consts.tile([1, 128], F32)
        nc.vector.tensor_mul(vnm, vals[:, 240:368], mn)
        nc.vector.tensor_mul(vsm, vals[:, 368:496], ms)

        s = consts.tile([1, 128], F32)
        nc.vector.memset(s, 1.0)
        s3 = s.rearrange("o (g t) -> o g t", t=TG)
        nc.vector.tensor_add(s3[:, 0:15, :], s3[:, 0:15, :], cview(vals, 0, 120, 15))
        nc.vector.tensor_add(s3[:, 1:16, :], s3[:, 1:16, :], cview(vals, 120, 120, 15))
        nc.vector.tensor_add(s, s, vnm[:])
        nc.vector.tensor_add(s, s, vsm[:])
        recip = consts.tile([1, 128], F32)
        nc.vector.reciprocal(recip, s)
        r3 = recip.rearrange("o (g t) -> o g t", t=TG)

        ve = consts.tile([1, 120], F32)
        vw = consts.tile([1, 120], F32)
        vn = consts.tile([1, 128], F32)
        vs = consts.tile([1, 128], F32)
        nc.vector.tensor_mul(cview(ve, 0, 120, 15), cview(vals, 0, 120, 15), r3[:, 0:15, :])
        nc.vector.tensor_mul(cview(vw, 0, 120, 15), cview(vals, 120, 120, 15), r3[:, 1:16, :])
        nc.vector.tensor_mul(vn, vnm[:], recip[:])
        nc.vector.tensor_mul(vs, vsm[:], recip[:])

        # ---- V table: (128 partitions) x (5 offsets) x (8 blocks) ----
        # offsets: 0:-256(N) 1:-4(W) 2:diag 3:+4(E) 4:+256(S)
        v = consts.tile([128, 5, TG], F32)
        nc.vector.memset(v, 0.0)
        nc.vector.memset(v[:, 2, :], 1.0)
        with nc.allow_non_contiguous_dma(reason="tiny per-partition scatter"):
            nc.gpsimd.dma_start(out=v[0:61:4, 2, :], in_=r3[:])
            nc.gpsimd.dma_start(
                out=v[0:61:4, 0, :], in_=vn.rearrange("o (g t) -> o g t", t=TG)
            )
            nc.gpsimd.dma_start(
                out=v[0:61:4, 4, :], in_=vs.rearrange("o (g t) -> o g t", t=TG)
            )
            nc.gpsimd.dma_start(
                out=v[0:57:4, 3, :], in_=ve.rearrange("o (g t) -> o g t", t=TG)
            )
            nc.gpsimd.dma_start(
                out=v[4:61:4, 1, :], in_=vw.rearrange("o (g t) -> o g t", t=TG)
            )

        # ---- build all 8 patch tiles at once (5 shifted diagonals scaled
        # by V) with stride-0 broadcast APs: 7 wide DVE ops (~4us) instead
        # of 64 per-tile ops, so pall is ready right after the MLP ----
        pall = consts.tile([128, TG, 640], F32)
        tmp = consts.tile([128, TG, 128], F32)
        nc.vector.memset(pall[:, :, 128:512], 0.0)
        ib = ident.unsqueeze(1).broadcast_to((128, TG, 128))

        def vb(k):
            return v[:, k, :].unsqueeze(2).broadcast_to((128, TG, 128))

        nc.vector.tensor_mul(pall[:, :, 0:128], ib, vb(0))
        nc.vector.tensor_mul(pall[:, :, 512:640], ib, vb(4))
        nc.vector.tensor_mul(pall[:, :, 252:380], ib, vb(1))
        nc.vector.tensor_mul(tmp, ib, vb(2))
        nc.vector.tensor_add(pall[:, :, 256:384], pall[:, :, 256:384], tmp[:])
        nc.vector.tensor_mul(tmp, ib, vb(3))
        nc.vector.tensor_add(pall[:, :, 260:388], pall[:, :, 260:388], tmp[:])

        # ---- the deferred patch-window writes (MLP done ~25 us into an
        # ~83 us stream), then the two widest bodies close the stream.
        # The patches are completion-rotation gated (~5 us apart at the
        # tail), so the big bodies MUST come after them: ending the FIFO
        # on 9 rotation-gated small DMAs drains the queue dry and
        # serializes them at ~5 us each (+17 us, measured). ----
        nc.sync.dma_start(out=rows_of(0)[:, 3840:HW], in_=pall[:, 0, 0:256])
        nc.sync.dma_start(out=rows_of(0)[:, 0:384], in_=pall[:, 0, 256:640])
        for t in range(1, TG):
            lb = 2 * t
            w0 = 128 * lb - 256
            nc.sync.dma_start(out=rows_of(lb)[:, w0 : w0 + 640], in_=pall[:, t, :])
        body(3)
        body(1)
    nc.compile()  # bacc register allocation — required before NEFF compile
    return nc


_NC_CACHE = None


def _get_nc():
    global _NC_CACHE
    if _NC_CACHE is None:
        _NC_CACHE = _build_nc()
    return _NC_CACHE


def kernel(**inputs) -> np.ndarray:
    global LAST_RESULTS
    features = np.ascontiguousarray(np.asarray(inputs["features"], dtype=np.float32))
    class_idx = int(np.asarray(inputs["class_idx"]))
    Hv = int(np.asarray(inputs["H"]))
    Wv = int(np.asarray(inputs["W"]))
    gs = int(np.asarray(inputs["grid_size"]))
    assert (Hv, Wv, gs) == (H, W, G), (Hv, Wv, gs)
    emb_table = np.asarray(inputs["emb_table"], dtype=np.float32)
    w1 = np.ascontiguousarray(np.asarray(inputs["W1"], np.float32)[class_idx])
    b1 = np.asarray(inputs["b1"], np.float32)[class_idx]
    w2 = np.ascontiguousarray(np.asarray(inputs["W2"], np.float32)[class_idx])
    b2 = np.asarray(inputs["b2"], np.float32)[class_idx]
    w3 = np.ascontiguousarray(np.asarray(inputs["W3"], np.float32)[class_idx])
    b3 = np.asarray(inputs["b3"], np.float32)[class_idx]

    # grid embeddings: rows gi*64+gj for gi,gj in {0,4,...,60}
    emb4 = np.ascontiguousarray(
        emb_table[: HW].reshape(H, W, 128)[::4, ::4]
    )  # (16,16,128)
    featg = features[:, :, ::4, ::4]  # (B, 256, 16, 16) strided view

    # class-independent part of the packed input block (bf16) + fp32 biases
    import ml_dtypes

    base = np.zeros((128, NCOL), ml_dtypes.bfloat16)
    base[:, C_W1:C_W2] = w1.reshape(5, 128, H1).transpose(1, 0, 2).reshape(128, 1280)
    base[:, C_W2:C_FT] = w2.reshape(2, 128, H2).transpose(1, 0, 2).reshape(128, 256)
    base[:, C_W3] = w3[:, 0]
    b12c = np.zeros((128, 3), np.float32)
    b12c[:, 0:2] = b1.reshape(2, 128).T
    b12c[:, 2] = b2

    in_maps = []
    for c in range(8):
        bb, hh = c // 2, c % 2
        # halo rows: local t=0 is north halo, t=1..8 own, t=9 south halo
        gus = [8 * hh - 1] + list(range(8 * hh, 8 * hh + 8)) + [8 * hh + 8]
        feat_core = np.zeros((D, 10, G), np.float32)
        emb_core = np.zeros((10 * G, 128), np.float32)
        for i, gu in enumerate(gus):
            if 0 <= gu < G:
                feat_core[:, i, :] = featg[bb, :, gu, :]
                emb_core[i * G : (i + 1) * G, :] = emb4[gu]
        inp_core = base.copy()
        inp_core[:, C_FT:C_EM] = (
            feat_core.reshape(2, 128, 160).transpose(1, 0, 2).reshape(128, 320)
        )
        inp_core[:, C_EM:C_W3] = 0.5 * emb_core.T
        msk_core = np.ones((1, 257), np.float32)
        # (g,t)-major: t=0 rows sit at indices g*8+0, t=7 at g*8+7
        if hh == 0:
            msk_core[0, 0:128:8] = 0.0  # no north neighbor on grid row 0
        else:
            msk_core[0, 135:256:8] = 0.0  # no south neighbor on grid row 15
        msk_core[0, 256] = b3[0]
        in_maps.append({"inp": inp_core, "b12": b12c, "msk": msk_core})

    nc = _get_nc()
    res = run_bass_kernel_spmd(nc, in_maps, core_ids=list(range(8)))
    LAST_RESULTS = res

    out = np.empty((B, HW, HW), np.float32)
    for c in range(8):
        bb, hh = c // 2, c % 2
        shard = res.results[c]["a"]
        if hh:
            shard = np.roll(shard, 2048, axis=1)
        out[bb, 2048 * hh : 2048 * (hh + 1), :] = shard
    return out
